# revision 1
# baseline (speedup 1.0000x reference)
"""Bloom self-attention (fused QKV + causal softmax attention) on 8 TRN2 cores.

Sharding: core c handles batch b=c//2 and head-group hg=c%2 (8 of 16 heads).
Each core computes QKV projection for its columns of W, then causal attention
for its 8 heads, writing out[s, 1024] (fp32). Host transposes/casts/slices
inputs and gathers outputs.

Layout notes (per core, on device):
  xt   [16,128,2048] bf16  : X_b^T d-tiles (d on partitions)
  wq/wk[8,128,2048]  bf16  : per head, stationary W tiles (partition-major)
  wv   [2,128,8192]  bf16  : per head-quad, 4 heads' v columns batched
  bq/bk[128,8]        f32  : per-partition bias columns per head
  bvq  [2,128,520]    bf16 : v-bias rows replicated + interleaved 1.0 columns
                             (130-stride: per quad-head 128 v cols, a ones col,
                              a zero pad col) -> attn@[v|1] yields rowsums
  mask [128,896]      bf16 : causal staircase; slice [:, 384-r:384-r+512] is
                             the 0/1 mask for a diagonal block at offset r

The emission order software-pipelines PE-dense work (QKV/V matmul chunks)
against ACT-bound attention chunks of the previous head, so the Tile
scheduler can fill exp-wait PE gaps with projection matmuls.
"""

import math
from contextlib import ExitStack

import numpy as np
import ml_dtypes

import concourse.mybir as mybir
import concourse.tile as tile
from concourse import bacc
from concourse.bass_utils import run_bass_kernel_spmd

B, S, D = 4, 2048, 2048
H, HD = 16, 128
N_CORES = 8
DT = D // 128   # 16 d-tiles
NQB = S // 512  # 4 q-blocks
SCALE = 1.0 / math.sqrt(HD)

BF16 = mybir.dt.bfloat16
F32 = mybir.dt.float32
BF16_NP = ml_dtypes.bfloat16


def build_nc(repeat: int = 1):
    nc = bacc.Bacc(
        "TRN2",
        target_bir_lowering=False,
        debug=False,
        enable_asserts=False,
        num_devices=N_CORES,
    )
    xt_d = nc.dram_tensor("xt", [DT, 128, S], BF16, kind="ExternalInput")
    wq_d = nc.dram_tensor("wq", [8, 128, DT * 128], BF16, kind="ExternalInput")
    wk_d = nc.dram_tensor("wk", [8, 128, DT * 128], BF16, kind="ExternalInput")
    wv_d = nc.dram_tensor("wv", [2, 128, DT * 512], BF16, kind="ExternalInput")
    bq_d = nc.dram_tensor("bq", [128, 8], F32, kind="ExternalInput")
    bk_d = nc.dram_tensor("bk", [128, 8], F32, kind="ExternalInput")
    bvq_d = nc.dram_tensor("bvq", [2, 128, 520], BF16, kind="ExternalInput")
    mask_d = nc.dram_tensor("mask", [128, 896], BF16, kind="ExternalInput")
    out_d = nc.dram_tensor("out", [S, 1024], F32, kind="ExternalOutput")

    with ExitStack() as ctx:
        tc = ctx.enter_context(tile.TileContext(nc))
        singles = ctx.enter_context(tc.tile_pool(name="singles", bufs=1))
        wqk_pool = ctx.enter_context(tc.tile_pool(name="wqk", bufs=2))
        wv_pool = ctx.enter_context(tc.tile_pool(name="wvp", bufs=2))
        qk_pool = ctx.enter_context(tc.tile_pool(name="qk", bufs=2))
        v4_pool = ctx.enter_context(tc.tile_pool(name="v4", bufs=2))
        p_pool = ctx.enter_context(tc.tile_pool(name="pp", bufs=24))
        o_pool = ctx.enter_context(tc.tile_pool(name="op", bufs=8))
        r_pool = ctx.enter_context(tc.tile_pool(name="rp", bufs=8))
        ps_big = ctx.enter_context(tc.tile_pool(name="ps_big", bufs=7, space="PSUM"))
        ps_out = ctx.enter_context(tc.tile_pool(name="ps_out", bufs=1, space="PSUM"))

        # ---- resident constants (loaded once) ----
        # Interleave wv(0) chunk DMAs with the xt tile DMAs: the first
        # (dt-outer) v-chunk consumes exactly wv[:, dt*512:...]+xt[dt] per
        # step, so the PE ramp starts as soon as the first pair lands.
        wv_first = wv_pool.tile([128, DT * 512], BF16, tag="wv")
        xt = []
        for dt in range(DT):
            nc.sync.dma_start(
                out=wv_first[:, dt * 512 : (dt + 1) * 512],
                in_=wv_d.ap()[0, :, dt * 512 : (dt + 1) * 512],
            )
            t = singles.tile([128, S], BF16, tag=f"xt{dt}")
            nc.sync.dma_start(out=t[:], in_=xt_d.ap()[dt, :, :])
            xt.append(t)
        mask = singles.tile([128, 896], BF16, tag="mask")
        nc.sync.dma_start(out=mask[:], in_=mask_d.ap())
        bvq = singles.tile([128, 2 * 520], BF16, tag="bvq")
        for g in range(2):
            nc.sync.dma_start(
                out=bvq[:, g * 520 : (g + 1) * 520], in_=bvq_d.ap()[g, :, :]
            )
        bq = singles.tile([128, 8], F32, tag="bq")
        nc.sync.dma_start(out=bq[:], in_=bq_d.ap())
        bk = singles.tile([128, 8], F32, tag="bk")
        nc.sync.dma_start(out=bk[:], in_=bk_d.ap())
        # prewarm the ACT exp table set (~2.7us PSEUDO_LOAD on first Exp)
        # while the startup DMAs run, instead of inside the first attention
        # chain
        warm = singles.tile([128, 1], F32, tag="warm")
        nc.vector.memset(warm[:], 0.0)
        nc.scalar.activation(warm[:], warm[:], mybir.ActivationFunctionType.Exp)

        for _rep in range(repeat):
            # per-rep state: tiles keyed by quad / head
            v4s = {}     # g -> [16 v4 tiles]
            wv_gs = {}   # g -> wv tile
            qks = {}     # h -> (qT, kT, wq_h, wk_h)

            def v_start(g):
                if g == 0 and _rep == 0:
                    wv_g = wv_first
                else:
                    wv_g = wv_pool.tile([128, DT * 512], BF16, tag="wv")
                    nc.sync.dma_start(out=wv_g[:], in_=wv_d.ap()[g, :, :])
                wv_gs[g] = wv_g
                v4s[g] = []

            def v_chunk(g, sts):
                """v4[st] = X @ Wv_quad + bv (+ interleaved ones cols).

                dt-outer over the st group so each xt[dt] tile is consumed as
                soon as its DMA lands (matters for the startup ramp)."""
                wv_g = wv_gs[g]
                sts = list(sts)
                psvs = []
                for st in sts:
                    psv = ps_big.tile([128, 512], F32, tag="ps_big")
                    psvs.append(psv)
                for dt in range(DT):
                    for st, psv in zip(sts, psvs):
                        nc.tensor.matmul(
                            psv[:],
                            lhsT=xt[dt][:, st * 128 : (st + 1) * 128],
                            rhs=wv_g[:, dt * 512 : (dt + 1) * 512],
                            start=(dt == 0),
                            stop=(dt == DT - 1),
                        )
                for st, psv in zip(sts, psvs):
                    v4t = v4_pool.tile([128, 520], BF16, tag=f"v4_{st}")
                    nc.vector.tensor_copy(v4t[:], bvq[:, g * 520 : (g + 1) * 520])
                    dst = v4t[:].rearrange("p (q c) -> p q c", q=4)[:, :, 0:128]
                    src = psv[:].rearrange("p (q c) -> p q c", q=4)
                    nc.vector.tensor_add(dst, dst, src)
                    v4s[g].append(v4t)

            def qkv_start(h):
                wq_h = wqk_pool.tile([128, DT * 128], BF16, tag="wq")
                nc.sync.dma_start(out=wq_h[:], in_=wq_d.ap()[h, :, :])
                wk_h = wqk_pool.tile([128, DT * 128], BF16, tag="wk")
                nc.sync.dma_start(out=wk_h[:], in_=wk_d.ap()[h, :, :])
                qT = qk_pool.tile([128, S], BF16, tag="qT")
                kT = qk_pool.tile([128, S], BF16, tag="kT")
                qks[h] = (qT, kT, wq_h, wk_h)

            def qkv_chunk(h, sb):
                """qT/kT columns for s-block sb of head h."""
                qT, kT, wq_h, wk_h = qks[h]
                for w_h, dest, bias in ((wq_h, qT, bq), (wk_h, kT, bk)):
                    psx = ps_big.tile([128, 512], F32, tag="ps_big")
                    for dt in range(DT):
                        nc.tensor.matmul(
                            psx[:],
                            lhsT=w_h[:, dt * 128 : (dt + 1) * 128],
                            rhs=xt[dt][:, sb * 512 : (sb + 1) * 512],
                            start=(dt == 0),
                            stop=(dt == DT - 1),
                        )
                    nc.vector.tensor_scalar_add(
                        dest[:, sb * 512 : (sb + 1) * 512], psx[:], bias[:, h : h + 1]
                    )

            attn_ps = {}  # (h, qb) -> [(p_tile, off)]

            def attn_scores(h, qb, lo=0, hi=None):
                """Scores + exp (+causal mask) for q-block qb of head h.

                Diagonal k-tiles are trimmed to their live width: tile kt
                covers q_local in [off, 512) with off = max(kt*128-qb*512, 0).
                """
                qT, kT = qks[h][0], qks[h][1]
                n_kt = 4 * qb + 4
                if hi is None:
                    hi = n_kt
                ps = attn_ps.setdefault((h, qb), [])
                for kt in range(lo, hi):
                    r = kt * 128 - qb * 512
                    off = max(r, 0)
                    nw = 512 - off
                    pss = ps_big.tile([128, 512], F32, tag="ps_big")
                    nc.tensor.matmul(
                        pss[:, 0:nw],
                        lhsT=kT[:, kt * 128 : (kt + 1) * 128],
                        rhs=qT[:, qb * 512 + off : (qb + 1) * 512],
                        start=True,
                        stop=True,
                    )
                    p_sb = p_pool.tile([128, 512], BF16, tag="p")
                    nc.scalar.activation(
                        p_sb[:, 0:nw], pss[:, 0:nw],
                        mybir.ActivationFunctionType.Exp, scale=SCALE,
                    )
                    if r >= 0:  # diagonal block: apply causal 0/1 mask
                        nc.vector.tensor_mul(
                            p_sb[:, 0:nw], p_sb[:, 0:nw], mask[:, 384 : 384 + nw]
                        )
                    ps.append((p_sb, off))

            def attn_out(h, qb):
                """attn @ [v|1], normalize, and store, for q-block qb."""
                g, hq = h // 4, h % 4
                v4 = v4s[g]
                ps = attn_ps.pop((h, qb))
                for j in range(4):
                    poj = ps_out.tile([128, 129], F32, tag="po")
                    last_kt = 4 * qb + j  # causality: kt*128 <= qb*512 + j*128
                    for kt in range(last_kt + 1):
                        p_sb, off = ps[kt]
                        nc.tensor.matmul(
                            poj[:],
                            lhsT=p_sb[:, j * 128 - off : j * 128 - off + 128],
                            rhs=v4[kt][:, hq * 130 : hq * 130 + 129],
                            start=(kt == 0),
                            stop=(kt == last_kt),
                        )
                    recip = r_pool.tile([128, 1], F32, tag="recip")
                    nc.vector.reciprocal(recip[:], poj[:, 128:129])
                    o_sb = o_pool.tile([128, 128], F32, tag="o")
                    nc.vector.tensor_scalar_mul(o_sb[:], poj[:, 0:128], recip[:])
                    nc.sync.dma_start(
                        out=out_d.ap()[
                            qb * 512 + j * 128 : qb * 512 + (j + 1) * 128,
                            h * 128 : (h + 1) * 128,
                        ],
                        in_=o_sb[:],
                    )

            # ---- software-pipelined emission ----
            # Each step pairs a PE-dense item (4 chunks) with the attention of
            # an already-projected head: scores(qb) -> pe chunk -> out(qb), so
            # exps for qb run on ACT while PE does projection matmuls.
            def v_item(g):
                v_start(g)
                return [lambda i=i: v_chunk(g, range(4 * i, 4 * i + 4))
                        for i in range(4)]

            def qkv_item(h):
                qkv_start(h)
                return [lambda sb=sb: qkv_chunk(h, sb) for sb in range(NQB)]

            steps = [
                (lambda: v_item(0), None),
                (lambda: qkv_item(0), None),
                (lambda: qkv_item(1), 0),
                (lambda: qkv_item(2), 1),
                (lambda: qkv_item(3), 2),
                (lambda: qkv_item(4), 3),
                (lambda: v_item(1), None),
                (lambda: qkv_item(5), 4),
                (lambda: qkv_item(6), 5),
                (lambda: qkv_item(7), 6),
                (None, 7),
            ]
            for pe_item, h_attn in steps:
                pe_chunks = pe_item() if pe_item is not None else [None] * NQB
                _lo = {0: 0, 1: 2, 2: 4, 3: 5}
                for i in range(NQB):
                    if h_attn is not None:
                        attn_scores(h_attn, i, lo=_lo[i])
                    if pe_chunks[i] is not None:
                        pe_chunks[i]()
                    if h_attn is not None:
                        if i + 1 < NQB and _lo[i + 1] > 0:
                            attn_scores(h_attn, i + 1, lo=0, hi=_lo[i + 1])
                        attn_out(h_attn, i)
    nc.compile()
    return nc


def make_in_maps(hidden_states, W, b):
    """Host-side sharding: slice/transpose/cast inputs per core."""
    X = np.asarray(hidden_states, dtype=np.float32)
    Wf = np.asarray(W, dtype=np.float32).reshape(D, D, 3)
    bf = np.asarray(b, dtype=np.float32).reshape(D, 3)

    # causal staircase mask: mask[p, c] = 1 if c >= p + 384
    cols = np.arange(896)[None, :]
    rows = np.arange(128)[:, None]
    mask = (cols >= rows + 384).astype(BF16_NP)

    in_maps = []
    for c in range(N_CORES):
        bcore, hg = c // 2, c % 2
        dm0 = hg * 1024
        xt = np.ascontiguousarray(X[bcore].T).reshape(DT, 128, S).astype(BF16_NP)
        wq = np.ascontiguousarray(
            Wf[:, dm0 : dm0 + 1024, 0].reshape(DT, 128, 8, 128).transpose(2, 1, 0, 3)
        ).reshape(8, 128, DT * 128).astype(BF16_NP)
        wk = np.ascontiguousarray(
            Wf[:, dm0 : dm0 + 1024, 2].reshape(DT, 128, 8, 128).transpose(2, 1, 0, 3)
        ).reshape(8, 128, DT * 128).astype(BF16_NP)
        wv = np.ascontiguousarray(
            Wf[:, dm0 : dm0 + 1024, 1].reshape(DT, 128, 2, 512).transpose(2, 1, 0, 3)
        ).reshape(2, 128, DT * 512).astype(BF16_NP)
        bq = np.ascontiguousarray(
            bf[dm0 : dm0 + 1024, 0].reshape(8, 128).T
        ).astype(np.float32)
        bk = np.ascontiguousarray(
            bf[dm0 : dm0 + 1024, 2].reshape(8, 128).T
        ).astype(np.float32)
        bv = bf[dm0 : dm0 + 1024, 1].reshape(2, 4, 128)
        bvq = np.zeros((2, 128, 520), dtype=BF16_NP)
        for g in range(2):
            for hq in range(4):
                bvq[g, :, hq * 130 : hq * 130 + 128] = bv[g, hq][None, :].astype(
                    BF16_NP
                )
                bvq[g, :, hq * 130 + 128] = BF16_NP(1.0)
        in_maps.append(
            {
                "xt": xt, "wq": wq, "wk": wk, "wv": wv,
                "bq": bq, "bk": bk, "bvq": bvq, "mask": mask,
            }
        )
    return in_maps


def gather_out(results):
    out = np.empty((B, S, D), dtype=np.float32)
    for c in range(N_CORES):
        bcore, hg = c // 2, c % 2
        out[bcore][:, hg * 1024 : hg * 1024 + 1024] = results[c]["out"]
    return out


_CACHED_NC = None


def kernel(hidden_states, W, b):
    global _CACHED_NC
    if _CACHED_NC is None:
        _CACHED_NC = build_nc()
    in_maps = make_in_maps(hidden_states, W, b)
    res = run_bass_kernel_spmd(_CACHED_NC, in_maps, core_ids=list(range(N_CORES)))
    return gather_out(res.results)



# revision 2
# speedup vs baseline: 1.1836x; 1.1836x over previous
"""Bloom self-attention (fused QKV + causal softmax attention) on 8 TRN2 cores.

Sharding: core c handles batch b=c//2 and head-group hg=c%2 (8 of 16 heads).
Each core computes QKV projection for its columns of W, then causal attention
for its 8 heads, writing out[s, 1024] (fp32). Host transposes/casts/slices
inputs and gathers outputs.

v2: QKV projections run as fp8(e4m3) DoubleRow matmuls — 2 contraction
k-tiles per instruction at 0.5 cycles/row, 4x bf16 PE throughput per pass.
To reach bf16-level accuracy each operand is split into fp8 hi+lo planes and
three passes accumulate (hi*hi + lo*hi + hi*lo); W is pre-scaled by 128 on
the host so the W lo-plane stays above the fp8 denormal threshold (raw
|W|~0.02 residuals would flush to zero). The scale cancels: q,k are 128x
(scores exp scale absorbs 128^2), v is 128x and the rowsum ones-column is
128.0 so normalization divides it out. Attention (scores/exp/mask/attn@v)
stays bf16, identical to the baseline structure.

Layout notes (per core, on device):
  xth/xtl [8,128,2*2048] fp8 : X^T hi/lo planes, dt-pair tiles
                               [p, j*2048+s] = plane(X[s, (2t2+j)*128+p])
  wqh/wql/wkh/wkl [8,128,2048] fp8 : per head, [p, t2*256+j*128+m]
  wvh/wvl [2,128,8192] fp8   : per group, [p, t2*1024+j*512+c]
  bq/bk  [128,8] f32         : per-partition bias columns per head (x128)
  bvq    [2,128,520] bf16    : v-bias rows (x128) + 128.0 ones columns
  mask   [128,896] bf16      : causal staircase (as baseline)
"""

import math
from contextlib import ExitStack

import numpy as np
import ml_dtypes

import concourse.mybir as mybir
import concourse.tile as tile
from concourse import bacc
from concourse.bass_utils import run_bass_kernel_spmd

B, S, D = 4, 2048, 2048
H, HD = 16, 128
N_CORES = 8
DT = D // 128   # 16 d-tiles
T2 = DT // 2    # 8 dt-pairs for DoubleRow
NQB = S // 512  # 4 q-blocks
WS = 128.0      # host pre-scale on W/b (keeps fp8 lo-planes out of denormals)
SCALE = 1.0 / (math.sqrt(HD) * WS * WS)

BF16 = mybir.dt.bfloat16
F8 = mybir.dt.float8e4
F32 = mybir.dt.float32
BF16_NP = ml_dtypes.bfloat16
F8_NP = ml_dtypes.float8_e4m3
DR = mybir.MatmulPerfMode.DoubleRow


def build_nc(repeat: int = 1):
    nc = bacc.Bacc(
        "TRN2",
        target_bir_lowering=False,
        debug=False,
        enable_asserts=False,
        num_devices=N_CORES,
    )
    xth_d = nc.dram_tensor("xth", [T2, 128, 2 * S], F8, kind="ExternalInput")
    xtl_d = nc.dram_tensor("xtl", [T2, 128, 2 * S], F8, kind="ExternalInput")
    wqh_d = nc.dram_tensor("wqh", [8, 128, DT * 128], F8, kind="ExternalInput")
    wql_d = nc.dram_tensor("wql", [8, 128, DT * 128], F8, kind="ExternalInput")
    wkh_d = nc.dram_tensor("wkh", [8, 128, DT * 128], F8, kind="ExternalInput")
    wkl_d = nc.dram_tensor("wkl", [8, 128, DT * 128], F8, kind="ExternalInput")
    wvh_d = nc.dram_tensor("wvh", [2, 128, DT * 512], F8, kind="ExternalInput")
    wvl_d = nc.dram_tensor("wvl", [2, 128, DT * 512], F8, kind="ExternalInput")
    bq_d = nc.dram_tensor("bq", [128, 8], F32, kind="ExternalInput")
    bk_d = nc.dram_tensor("bk", [128, 8], F32, kind="ExternalInput")
    bvq_d = nc.dram_tensor("bvq", [2, 128, 520], BF16, kind="ExternalInput")
    mask_d = nc.dram_tensor("mask", [128, 896], BF16, kind="ExternalInput")
    out_d = nc.dram_tensor("out", [S, 1024], F32, kind="ExternalOutput")

    with ExitStack() as ctx:
        tc = ctx.enter_context(tile.TileContext(nc))
        singles = ctx.enter_context(tc.tile_pool(name="singles", bufs=1))
        wqk_pool = ctx.enter_context(tc.tile_pool(name="wqk", bufs=2))
        wv_pool = ctx.enter_context(tc.tile_pool(name="wvp", bufs=2))
        qk_pool = ctx.enter_context(tc.tile_pool(name="qk", bufs=2))
        v4_pool = ctx.enter_context(tc.tile_pool(name="v4", bufs=2))
        p_pool = ctx.enter_context(tc.tile_pool(name="pp", bufs=24))
        o_pool = ctx.enter_context(tc.tile_pool(name="op", bufs=8))
        r_pool = ctx.enter_context(tc.tile_pool(name="rp", bufs=8))
        ps_big = ctx.enter_context(tc.tile_pool(name="ps_big", bufs=7, space="PSUM"))
        ps_out = ctx.enter_context(tc.tile_pool(name="ps_out", bufs=1, space="PSUM"))

        # ---- resident constants (loaded once) ----
        # Interleave wv(0) chunk DMAs with the xt tile DMAs: the first
        # (t2-outer) v-chunk consumes exactly wv[:, t2*1024:...]+xt[t2] per
        # step, so the PE ramp starts as soon as the first pair lands.
        wvh_first = wv_pool.tile([128, DT * 512], F8, tag="wvh")
        wvl_first = wv_pool.tile([128, DT * 512], F8, tag="wvl")
        xth = []
        xtl = []
        for t2 in range(T2):
            nc.sync.dma_start(
                out=wvh_first[:, t2 * 1024 : (t2 + 1) * 1024],
                in_=wvh_d.ap()[0, :, t2 * 1024 : (t2 + 1) * 1024],
            )
            nc.sync.dma_start(
                out=wvl_first[:, t2 * 1024 : (t2 + 1) * 1024],
                in_=wvl_d.ap()[0, :, t2 * 1024 : (t2 + 1) * 1024],
            )
            th = singles.tile([128, 2 * S], F8, tag=f"xth{t2}")
            nc.sync.dma_start(out=th[:], in_=xth_d.ap()[t2, :, :])
            xth.append(th)
            tl = singles.tile([128, 2 * S], F8, tag=f"xtl{t2}")
            nc.sync.dma_start(out=tl[:], in_=xtl_d.ap()[t2, :, :])
            xtl.append(tl)
        mask = singles.tile([128, 896], BF16, tag="mask")
        nc.sync.dma_start(out=mask[:], in_=mask_d.ap())
        bvq = singles.tile([128, 2 * 520], BF16, tag="bvq")
        for g in range(2):
            nc.sync.dma_start(
                out=bvq[:, g * 520 : (g + 1) * 520], in_=bvq_d.ap()[g, :, :]
            )
        bq = singles.tile([128, 8], F32, tag="bq")
        nc.sync.dma_start(out=bq[:], in_=bq_d.ap())
        bk = singles.tile([128, 8], F32, tag="bk")
        nc.sync.dma_start(out=bk[:], in_=bk_d.ap())
        # prewarm the ACT exp table set (~2.7us PSEUDO_LOAD on first Exp)
        warm = singles.tile([128, 1], F32, tag="warm")
        nc.vector.memset(warm[:], 0.0)
        nc.scalar.activation(warm[:], warm[:], mybir.ActivationFunctionType.Exp)

        def dr3(psum, xh_ap, xl_ap, wh_ap, wl_ap, t2, x_stationary):
            """Emit the 3 compensated fp8 DR passes for one t2 pair.

            x_stationary: X planes are lhsT (v-projection); else W planes
            are lhsT (q/k projection)."""
            first = t2 == 0
            last = t2 == T2 - 1
            if x_stationary:
                triples = [(xh_ap, wh_ap), (xl_ap, wh_ap), (xh_ap, wl_ap)]
            else:
                triples = [(wh_ap, xh_ap), (wh_ap, xl_ap), (wl_ap, xh_ap)]
            for i, (lhsT, rhs) in enumerate(triples):
                nc.tensor.matmul(
                    psum,
                    lhsT=lhsT,
                    rhs=rhs,
                    start=(first and i == 0),
                    stop=(last and i == 2),
                    perf_mode=DR,
                )

        for _rep in range(repeat):
            # per-rep state: tiles keyed by quad / head
            v4s = {}     # g -> [16 v4 tiles]
            wv_gs = {}   # g -> (wvh, wvl) tiles
            qks = {}     # h -> (qT, kT, (wqh, wql, wkh, wkl))

            def v_start(g):
                if g == 0 and _rep == 0:
                    wv_g = (wvh_first, wvl_first)
                else:
                    wvh_g = wv_pool.tile([128, DT * 512], F8, tag="wvh")
                    nc.sync.dma_start(out=wvh_g[:], in_=wvh_d.ap()[g, :, :])
                    wvl_g = wv_pool.tile([128, DT * 512], F8, tag="wvl")
                    nc.sync.dma_start(out=wvl_g[:], in_=wvl_d.ap()[g, :, :])
                    wv_g = (wvh_g, wvl_g)
                wv_gs[g] = wv_g
                v4s[g] = []

            def v_chunk(g, sts):
                """v4[st] = X @ Wv_quad + bv (+ interleaved ones cols).

                t2-outer over the st group so each xt[t2] tile is consumed as
                soon as its DMA lands (matters for the startup ramp)."""
                wvh_g, wvl_g = wv_gs[g]
                wvh_v = wvh_g[:].rearrange("p (t j c) -> p t j c", t=T2, j=2)
                wvl_v = wvl_g[:].rearrange("p (t j c) -> p t j c", t=T2, j=2)
                sts = list(sts)
                psvs = []
                for st in sts:
                    psv = ps_big.tile([128, 512], F32, tag="ps_big")
                    psvs.append(psv)
                for t2 in range(T2):
                    xh_v = xth[t2][:].rearrange("p (j s) -> p j s", j=2)
                    xl_v = xtl[t2][:].rearrange("p (j s) -> p j s", j=2)
                    for st, psv in zip(sts, psvs):
                        dr3(
                            psv[:],
                            xh_v[:, :, st * 128 : (st + 1) * 128],
                            xl_v[:, :, st * 128 : (st + 1) * 128],
                            wvh_v[:, t2],
                            wvl_v[:, t2],
                            t2,
                            x_stationary=True,
                        )
                for st, psv in zip(sts, psvs):
                    v4t = v4_pool.tile([128, 520], BF16, tag=f"v4_{st}")
                    nc.vector.tensor_copy(v4t[:], bvq[:, g * 520 : (g + 1) * 520])
                    dst = v4t[:].rearrange("p (q c) -> p q c", q=4)[:, :, 0:128]
                    src = psv[:].rearrange("p (q c) -> p q c", q=4)
                    nc.vector.tensor_add(dst, dst, src)
                    v4s[g].append(v4t)

            def qkv_start(h):
                wqh_h = wqk_pool.tile([128, DT * 128], F8, tag="wqh")
                nc.sync.dma_start(out=wqh_h[:], in_=wqh_d.ap()[h, :, :])
                wql_h = wqk_pool.tile([128, DT * 128], F8, tag="wql")
                nc.sync.dma_start(out=wql_h[:], in_=wql_d.ap()[h, :, :])
                wkh_h = wqk_pool.tile([128, DT * 128], F8, tag="wkh")
                nc.sync.dma_start(out=wkh_h[:], in_=wkh_d.ap()[h, :, :])
                wkl_h = wqk_pool.tile([128, DT * 128], F8, tag="wkl")
                nc.sync.dma_start(out=wkl_h[:], in_=wkl_d.ap()[h, :, :])
                qT = qk_pool.tile([128, S], BF16, tag="qT")
                kT = qk_pool.tile([128, S], BF16, tag="kT")
                qks[h] = (qT, kT, (wqh_h, wql_h, wkh_h, wkl_h))

            def qkv_chunk(h, sb):
                """qT/kT columns for s-block sb of head h."""
                qT, kT, (wqh_h, wql_h, wkh_h, wkl_h) = qks[h]
                for wh, wl, dest, bias in (
                    (wqh_h, wql_h, qT, bq),
                    (wkh_h, wkl_h, kT, bk),
                ):
                    wh_v = wh[:].rearrange("p (t j m) -> p t j m", t=T2, j=2)
                    wl_v = wl[:].rearrange("p (t j m) -> p t j m", t=T2, j=2)
                    psx = ps_big.tile([128, 512], F32, tag="ps_big")
                    for t2 in range(T2):
                        xh_v = xth[t2][:].rearrange("p (j s) -> p j s", j=2)
                        xl_v = xtl[t2][:].rearrange("p (j s) -> p j s", j=2)
                        dr3(
                            psx[:],
                            xh_v[:, :, sb * 512 : (sb + 1) * 512],
                            xl_v[:, :, sb * 512 : (sb + 1) * 512],
                            wh_v[:, t2],
                            wl_v[:, t2],
                            t2,
                            x_stationary=False,
                        )
                    nc.vector.tensor_scalar_add(
                        dest[:, sb * 512 : (sb + 1) * 512], psx[:], bias[:, h : h + 1]
                    )

            attn_ps = {}  # (h, qb) -> [(p_tile, off)]

            def attn_scores(h, qb, lo=0, hi=None):
                """Scores + exp (+causal mask) for q-block qb of head h.

                Diagonal k-tiles are trimmed to their live width: tile kt
                covers q_local in [off, 512) with off = max(kt*128-qb*512, 0).
                """
                qT, kT = qks[h][0], qks[h][1]
                n_kt = 4 * qb + 4
                if hi is None:
                    hi = n_kt
                ps = attn_ps.setdefault((h, qb), [])
                for kt in range(lo, hi):
                    r = kt * 128 - qb * 512
                    off = max(r, 0)
                    nw = 512 - off
                    pss = ps_big.tile([128, 512], F32, tag="ps_big")
                    nc.tensor.matmul(
                        pss[:, 0:nw],
                        lhsT=kT[:, kt * 128 : (kt + 1) * 128],
                        rhs=qT[:, qb * 512 + off : (qb + 1) * 512],
                        start=True,
                        stop=True,
                    )
                    p_sb = p_pool.tile([128, 512], BF16, tag="p")
                    nc.scalar.activation(
                        p_sb[:, 0:nw], pss[:, 0:nw],
                        mybir.ActivationFunctionType.Exp, scale=SCALE,
                    )
                    if r >= 0:  # diagonal block: apply causal 0/1 mask
                        nc.vector.tensor_mul(
                            p_sb[:, 0:nw], p_sb[:, 0:nw], mask[:, 384 : 384 + nw]
                        )
                    ps.append((p_sb, off))

            def attn_out(h, qb):
                """attn @ [v|1], normalize, and store, for q-block qb."""
                g, hq = h // 4, h % 4
                v4 = v4s[g]
                ps = attn_ps.pop((h, qb))
                for j in range(4):
                    poj = ps_out.tile([128, 129], F32, tag="po")
                    last_kt = 4 * qb + j  # causality: kt*128 <= qb*512 + j*128
                    for kt in range(last_kt + 1):
                        p_sb, off = ps[kt]
                        nc.tensor.matmul(
                            poj[:],
                            lhsT=p_sb[:, j * 128 - off : j * 128 - off + 128],
                            rhs=v4[kt][:, hq * 130 : hq * 130 + 129],
                            start=(kt == 0),
                            stop=(kt == last_kt),
                        )
                    recip = r_pool.tile([128, 1], F32, tag="recip")
                    nc.vector.reciprocal(recip[:], poj[:, 128:129])
                    o_sb = o_pool.tile([128, 128], F32, tag="o")
                    nc.vector.tensor_scalar_mul(o_sb[:], poj[:, 0:128], recip[:])
                    nc.sync.dma_start(
                        out=out_d.ap()[
                            qb * 512 + j * 128 : qb * 512 + (j + 1) * 128,
                            h * 128 : (h + 1) * 128,
                        ],
                        in_=o_sb[:],
                    )

            # ---- software-pipelined emission ----
            # Each step pairs a PE-dense item (4 chunks) with the attention of
            # an already-projected head: scores(qb) -> pe chunk -> out(qb), so
            # exps for qb run on ACT while PE does projection matmuls.
            def v_item(g):
                v_start(g)
                return [lambda i=i: v_chunk(g, range(4 * i, 4 * i + 4))
                        for i in range(4)]

            def qkv_item(h):
                qkv_start(h)
                return [lambda sb=sb: qkv_chunk(h, sb) for sb in range(NQB)]

            steps = [
                (lambda: v_item(0), None),
                (lambda: qkv_item(0), None),
                (lambda: qkv_item(1), 0),
                (lambda: qkv_item(2), 1),
                (lambda: qkv_item(3), 2),
                (lambda: qkv_item(4), 3),
                (lambda: v_item(1), None),
                (lambda: qkv_item(5), 4),
                (lambda: qkv_item(6), 5),
                (lambda: qkv_item(7), 6),
                (None, 7),
            ]
            for pe_item, h_attn in steps:
                pe_chunks = pe_item() if pe_item is not None else [None] * NQB
                _lo = {0: 0, 1: 2, 2: 4, 3: 5}
                for i in range(NQB):
                    if h_attn is not None:
                        attn_scores(h_attn, i, lo=_lo[i])
                    if pe_chunks[i] is not None:
                        pe_chunks[i]()
                    if h_attn is not None:
                        if i + 1 < NQB and _lo[i + 1] > 0:
                            attn_scores(h_attn, i + 1, lo=0, hi=_lo[i + 1])
                        attn_out(h_attn, i)
    nc.compile()
    return nc


def _hilo(x):
    hi = x.astype(F8_NP)
    lo = (x - hi.astype(np.float32)).astype(F8_NP)
    return hi, lo


def make_in_maps(hidden_states, W, b):
    """Host-side sharding: slice/transpose/cast inputs per core."""
    X = np.asarray(hidden_states, dtype=np.float32)
    Wf = np.asarray(W, dtype=np.float32).reshape(D, D, 3) * WS
    bf = np.asarray(b, dtype=np.float32).reshape(D, 3) * WS

    # causal staircase mask: mask[p, c] = 1 if c >= p + 384
    cols = np.arange(896)[None, :]
    rows = np.arange(128)[:, None]
    mask = (cols >= rows + 384).astype(BF16_NP)

    in_maps = []
    for c in range(N_CORES):
        bcore, hg = c // 2, c % 2
        dm0 = hg * 1024
        # X^T planes in dt-pair layout [t2][p][j*S+s]
        xt_f = np.ascontiguousarray(X[bcore].T)  # [D, S] f32
        xh, xl = _hilo(xt_f)
        xth = np.ascontiguousarray(
            xh.reshape(T2, 2, 128, S).transpose(0, 2, 1, 3)
        ).reshape(T2, 128, 2 * S)
        xtl = np.ascontiguousarray(
            xl.reshape(T2, 2, 128, S).transpose(0, 2, 1, 3)
        ).reshape(T2, 128, 2 * S)

        def w_planes(col_plane):
            # [D, 1024] -> [8 heads][128 p][t2*256 + j*128 + m]
            wh, wl = _hilo(col_plane)
            def lay(a):
                return np.ascontiguousarray(
                    a.reshape(T2, 2, 128, 8, 128).transpose(3, 2, 0, 1, 4)
                ).reshape(8, 128, DT * 128)
            return lay(wh), lay(wl)

        wqh, wql = w_planes(Wf[:, dm0 : dm0 + 1024, 0])
        wkh, wkl = w_planes(Wf[:, dm0 : dm0 + 1024, 2])

        # V: [D, 1024] -> [2 groups][128 p][t2*1024 + j*512 + c]
        vh, vl = _hilo(Wf[:, dm0 : dm0 + 1024, 1])
        def v_lay(a):
            return np.ascontiguousarray(
                a.reshape(T2, 2, 128, 2, 512).transpose(3, 2, 0, 1, 4)
            ).reshape(2, 128, DT * 512)
        wvh, wvl = v_lay(vh), v_lay(vl)

        bq = np.ascontiguousarray(
            bf[dm0 : dm0 + 1024, 0].reshape(8, 128).T
        ).astype(np.float32)
        bk = np.ascontiguousarray(
            bf[dm0 : dm0 + 1024, 2].reshape(8, 128).T
        ).astype(np.float32)
        bv = bf[dm0 : dm0 + 1024, 1].reshape(2, 4, 128)
        bvq = np.zeros((2, 128, 520), dtype=BF16_NP)
        for g in range(2):
            for hq in range(4):
                bvq[g, :, hq * 130 : hq * 130 + 128] = bv[g, hq][None, :].astype(
                    BF16_NP
                )
                bvq[g, :, hq * 130 + 128] = BF16_NP(WS)  # rowsum col: x128 like v
        in_maps.append(
            {
                "xth": xth, "xtl": xtl,
                "wqh": wqh, "wql": wql, "wkh": wkh, "wkl": wkl,
                "wvh": wvh, "wvl": wvl,
                "bq": bq, "bk": bk, "bvq": bvq, "mask": mask,
            }
        )
    return in_maps


def gather_out(results):
    out = np.empty((B, S, D), dtype=np.float32)
    for c in range(N_CORES):
        bcore, hg = c // 2, c % 2
        out[bcore][:, hg * 1024 : hg * 1024 + 1024] = results[c]["out"]
    return out


_CACHED_NC = None


def kernel(hidden_states, W, b):
    global _CACHED_NC
    if _CACHED_NC is None:
        _CACHED_NC = build_nc()
    in_maps = make_in_maps(hidden_states, W, b)
    res = run_bass_kernel_spmd(_CACHED_NC, in_maps, core_ids=list(range(N_CORES)))
    return gather_out(res.results)


# revision 25
# speedup vs baseline: 1.1838x; 1.0001x over previous
"""Bloom self-attention (fused QKV + causal softmax attention) on 8 TRN2 cores.

Sharding: core c handles batch b=c//2 and head-group hg=c%2 (8 of 16 heads).
Each core computes QKV projection for its columns of W, then causal attention
for its 8 heads, writing out[s, 1024] (fp32). Host transposes/casts/slices
inputs and gathers outputs.

v2: QKV projections run as fp8(e4m3) DoubleRow matmuls — 2 contraction
k-tiles per instruction at 0.5 cycles/row, 4x bf16 PE throughput per pass.
To reach bf16-level accuracy each operand is split into fp8 hi+lo planes and
three passes accumulate (hi*hi + lo*hi + hi*lo); W is pre-scaled by 128 on
the host so the W lo-plane stays above the fp8 denormal threshold (raw
|W|~0.02 residuals would flush to zero). The scale cancels: q,k are 128x
(scores exp scale absorbs 128^2), v is 128x and the rowsum ones-column is
128.0 so normalization divides it out. Attention (scores/exp/mask/attn@v)
stays bf16, identical to the baseline structure.

Layout notes (per core, on device):
  xth/xtl [8,128,2*2048] fp8 : X^T hi/lo planes, dt-pair tiles
                               [p, j*2048+s] = plane(X[s, (2t2+j)*128+p])
  wqh/wql/wkh/wkl [8,128,2048] fp8 : per head, [p, t2*256+j*128+m]
  wvh/wvl [2,128,8192] fp8   : per group, [p, t2*1024+j*512+c]
  bq/bk  [128,8] f32         : per-partition bias columns per head (x128)
  bvq    [2,128,520] bf16    : v-bias rows (x128) + 128.0 ones columns
  mask   [128,896] bf16      : causal staircase (as baseline)
"""

import math
from contextlib import ExitStack

import numpy as np
import ml_dtypes

import concourse.mybir as mybir
import concourse.tile as tile
from concourse import bacc
from concourse.bass_utils import run_bass_kernel_spmd

B, S, D = 4, 2048, 2048
H, HD = 16, 128
N_CORES = 8
DT = D // 128   # 16 d-tiles
T2 = DT // 2    # 8 dt-pairs for DoubleRow
NQB = S // 512  # 4 q-blocks
WS = 128.0      # host pre-scale on W/b (keeps fp8 lo-planes out of denormals)
SCALE = 1.0 / (math.sqrt(HD) * WS * WS)

BF16 = mybir.dt.bfloat16
F8 = mybir.dt.float8e4
F32 = mybir.dt.float32
BF16_NP = ml_dtypes.bfloat16
F8_NP = ml_dtypes.float8_e4m3
DR = mybir.MatmulPerfMode.DoubleRow


def build_nc(repeat: int = 1):
    nc = bacc.Bacc(
        "TRN2",
        target_bir_lowering=False,
        debug=False,
        enable_asserts=False,
        num_devices=N_CORES,
    )
    xth_d = nc.dram_tensor("xth", [T2, 128, 2 * S], F8, kind="ExternalInput")
    xtl_d = nc.dram_tensor("xtl", [T2, 128, 2 * S], F8, kind="ExternalInput")
    wqh_d = nc.dram_tensor("wqh", [8, 128, DT * 128], F8, kind="ExternalInput")
    wql_d = nc.dram_tensor("wql", [8, 128, DT * 128], F8, kind="ExternalInput")
    wkh_d = nc.dram_tensor("wkh", [8, 128, DT * 128], F8, kind="ExternalInput")
    wkl_d = nc.dram_tensor("wkl", [8, 128, DT * 128], F8, kind="ExternalInput")
    wvh_d = nc.dram_tensor("wvh", [2, 128, DT * 512], F8, kind="ExternalInput")
    wvl_d = nc.dram_tensor("wvl", [2, 128, DT * 512], F8, kind="ExternalInput")
    bq_d = nc.dram_tensor("bq", [128, 8], F32, kind="ExternalInput")
    bk_d = nc.dram_tensor("bk", [128, 8], F32, kind="ExternalInput")
    bvq_d = nc.dram_tensor("bvq", [2, 128, 520], BF16, kind="ExternalInput")
    mask_d = nc.dram_tensor("mask", [128, 896], BF16, kind="ExternalInput")
    out_d = nc.dram_tensor("out", [S, 1024], F32, kind="ExternalOutput")

    with ExitStack() as ctx:
        tc = ctx.enter_context(tile.TileContext(nc))
        singles = ctx.enter_context(tc.tile_pool(name="singles", bufs=1))
        wqk_pool = ctx.enter_context(tc.tile_pool(name="wqk", bufs=2))
        wv_pool = ctx.enter_context(tc.tile_pool(name="wvp", bufs=2))
        qk_pool = ctx.enter_context(tc.tile_pool(name="qk", bufs=2))
        v4_pool = ctx.enter_context(tc.tile_pool(name="v4", bufs=2))
        p_pool = ctx.enter_context(tc.tile_pool(name="pp", bufs=28))
        o_pool = ctx.enter_context(tc.tile_pool(name="op", bufs=4))
        r_pool = ctx.enter_context(tc.tile_pool(name="rp", bufs=8))
        ps_big = ctx.enter_context(tc.tile_pool(name="ps_big", bufs=7, space="PSUM"))
        ps_out = ctx.enter_context(tc.tile_pool(name="ps_out", bufs=1, space="PSUM"))

        # ---- resident constants (loaded once) ----
        # Interleave wv(0) chunk DMAs with the xt tile DMAs: the first
        # (t2-outer) v-chunk consumes exactly wv[:, t2*1024:...]+xt[t2] per
        # step, so the PE ramp starts as soon as the first pair lands.
        wvh_first = wv_pool.tile([128, DT * 512], F8, tag="wvh")
        wvl_first = wv_pool.tile([128, DT * 512], F8, tag="wvl")
        xth = []
        xtl = []
        for t2 in range(T2):
            nc.sync.dma_start(
                out=wvh_first[:, t2 * 1024 : (t2 + 1) * 1024],
                in_=wvh_d.ap()[0, :, t2 * 1024 : (t2 + 1) * 1024],
            )
            nc.sync.dma_start(
                out=wvl_first[:, t2 * 1024 : (t2 + 1) * 1024],
                in_=wvl_d.ap()[0, :, t2 * 1024 : (t2 + 1) * 1024],
            )
            th = singles.tile([128, 2 * S], F8, tag=f"xth{t2}")
            nc.sync.dma_start(out=th[:], in_=xth_d.ap()[t2, :, :])
            xth.append(th)
            tl = singles.tile([128, 2 * S], F8, tag=f"xtl{t2}")
            nc.sync.dma_start(out=tl[:], in_=xtl_d.ap()[t2, :, :])
            xtl.append(tl)
        mask = singles.tile([128, 896], BF16, tag="mask")
        nc.sync.dma_start(out=mask[:], in_=mask_d.ap())
        bvq = singles.tile([128, 2 * 520], BF16, tag="bvq")
        for g in range(2):
            nc.sync.dma_start(
                out=bvq[:, g * 520 : (g + 1) * 520], in_=bvq_d.ap()[g, :, :]
            )
        bq = singles.tile([128, 8], F32, tag="bq")
        nc.sync.dma_start(out=bq[:], in_=bq_d.ap())
        bk = singles.tile([128, 8], F32, tag="bk")
        nc.sync.dma_start(out=bk[:], in_=bk_d.ap())
        # prewarm the ACT exp table set (~2.7us PSEUDO_LOAD on first Exp)
        warm = singles.tile([128, 1], F32, tag="warm")
        nc.vector.memset(warm[:], 0.0)
        nc.scalar.activation(warm[:], warm[:], mybir.ActivationFunctionType.Exp)

        def dr3(psum, xh_ap, xl_ap, wh_ap, wl_ap, t2, x_stationary):
            """Emit the 3 compensated fp8 DR passes for one t2 pair.

            x_stationary: X planes are lhsT (v-projection); else W planes
            are lhsT (q/k projection)."""
            first = t2 == 0
            last = t2 == T2 - 1
            if x_stationary:
                triples = [(xh_ap, wh_ap), (xh_ap, wl_ap), (xl_ap, wh_ap)]
            else:
                triples = [(wh_ap, xh_ap), (wl_ap, xh_ap), (wh_ap, xl_ap)]
            for i, (lhsT, rhs) in enumerate(triples):
                nc.tensor.matmul(
                    psum,
                    lhsT=lhsT,
                    rhs=rhs,
                    start=(first and i == 0),
                    stop=(last and i == 2),
                    perf_mode=DR,
                )

        for _rep in range(repeat):
            # per-rep state: tiles keyed by quad / head
            v4s = {}     # g -> [16 v4 tiles]
            wv_gs = {}   # g -> (wvh, wvl) tiles
            qks = {}     # h -> (qT, kT, (wqh, wql, wkh, wkl))

            def v_start(g):
                if g == 0 and _rep == 0:
                    wv_g = (wvh_first, wvl_first)
                else:
                    wvh_g = wv_pool.tile([128, DT * 512], F8, tag="wvh")
                    nc.sync.dma_start(out=wvh_g[:], in_=wvh_d.ap()[g, :, :])
                    wvl_g = wv_pool.tile([128, DT * 512], F8, tag="wvl")
                    nc.sync.dma_start(out=wvl_g[:], in_=wvl_d.ap()[g, :, :])
                    wv_g = (wvh_g, wvl_g)
                wv_gs[g] = wv_g
                v4s[g] = []

            def v_chunk(g, sts):
                """v4[st] = X @ Wv_quad + bv (+ interleaved ones cols).

                t2-outer over the st group so each xt[t2] tile is consumed as
                soon as its DMA lands (matters for the startup ramp)."""
                wvh_g, wvl_g = wv_gs[g]
                wvh_v = wvh_g[:].rearrange("p (t j c) -> p t j c", t=T2, j=2)
                wvl_v = wvl_g[:].rearrange("p (t j c) -> p t j c", t=T2, j=2)
                sts = list(sts)
                psvs = []
                for st in sts:
                    psv = ps_big.tile([128, 512], F32, tag="ps_big")
                    psvs.append(psv)
                for t2 in range(T2):
                    xh_v = xth[t2][:].rearrange("p (j s) -> p j s", j=2)
                    xl_v = xtl[t2][:].rearrange("p (j s) -> p j s", j=2)
                    for st, psv in zip(sts, psvs):
                        dr3(
                            psv[:],
                            xh_v[:, :, st * 128 : (st + 1) * 128],
                            xl_v[:, :, st * 128 : (st + 1) * 128],
                            wvh_v[:, t2],
                            wvl_v[:, t2],
                            t2,
                            x_stationary=True,
                        )
                for st, psv in zip(sts, psvs):
                    v4t = v4_pool.tile([128, 520], BF16, tag=f"v4_{st}")
                    nc.vector.tensor_copy(v4t[:], bvq[:, g * 520 : (g + 1) * 520])
                    dst = v4t[:].rearrange("p (q c) -> p q c", q=4)[:, :, 0:128]
                    src = psv[:].rearrange("p (q c) -> p q c", q=4)
                    nc.vector.tensor_add(dst, dst, src)
                    v4s[g].append(v4t)

            def qkv_start(h):
                ws = []
                for nm, d in (
                    ("wqh", wqh_d), ("wql", wql_d),
                    ("wkh", wkh_d), ("wkl", wkl_d),
                ):
                    t = wqk_pool.tile([128, DT * 128], F8, tag=nm)
                    nc.sync.dma_start(out=t[:], in_=d.ap()[h, :, :])
                    ws.append(t)
                ws = tuple(ws)
                qT = qk_pool.tile([128, S], BF16, tag="qT")
                kT = qk_pool.tile([128, S], BF16, tag="kT")
                qks[h] = (qT, kT, ws)

            def qkv_chunk(h, sb):
                """qT/kT columns for s-block sb of head h."""
                qT, kT, (wqh_h, wql_h, wkh_h, wkl_h) = qks[h]
                for wh, wl, dest, bias in (
                    (wqh_h, wql_h, qT, bq),
                    (wkh_h, wkl_h, kT, bk),
                ):
                    wh_v = wh[:].rearrange("p (t j m) -> p t j m", t=T2, j=2)
                    wl_v = wl[:].rearrange("p (t j m) -> p t j m", t=T2, j=2)
                    psx = ps_big.tile([128, 512], F32, tag="ps_big")
                    for t2 in range(T2):
                        xh_v = xth[t2][:].rearrange("p (j s) -> p j s", j=2)
                        xl_v = xtl[t2][:].rearrange("p (j s) -> p j s", j=2)
                        dr3(
                            psx[:],
                            xh_v[:, :, sb * 512 : (sb + 1) * 512],
                            xl_v[:, :, sb * 512 : (sb + 1) * 512],
                            wh_v[:, t2],
                            wl_v[:, t2],
                            t2,
                            x_stationary=False,
                        )
                    nc.vector.tensor_scalar_add(
                        dest[:, sb * 512 : (sb + 1) * 512], psx[:], bias[:, h : h + 1]
                    )

            def merged_chunk(i):
                """Startup chunk: v0 sts 4i..4i+3 + qkv0 (q,k) for sb=i.

                Passes are slot-pipelined across t2 (A at slot s, C at s-1,
                B at s-2) so consumption tracks the per-t2 DMA delivery
                order (wvh, xth, wvl, xtl) with only 6 open PSUM groups."""
                wvh_g, wvl_g = wv_gs[0]
                wvh_v = wvh_g[:].rearrange("p (t j c) -> p t j c", t=T2, j=2)
                wvl_v = wvl_g[:].rearrange("p (t j c) -> p t j c", t=T2, j=2)
                qT, kT, (wqh_h, wql_h, wkh_h, wkl_h) = qks[0]
                wqh_v = wqh_h[:].rearrange("p (t j m) -> p t j m", t=T2, j=2)
                wql_v = wql_h[:].rearrange("p (t j m) -> p t j m", t=T2, j=2)
                wkh_v = wkh_h[:].rearrange("p (t j m) -> p t j m", t=T2, j=2)
                wkl_v = wkl_h[:].rearrange("p (t j m) -> p t j m", t=T2, j=2)
                sts = list(range(4 * i, 4 * i + 4))
                psvs = []
                for _st in sts:
                    psv = ps_big.tile([128, 512], F32, tag="ps_big")
                    psvs.append(psv)
                psq = ps_big.tile([128, 512], F32, tag="ps_big")
                psk = ps_big.tile([128, 512], F32, tag="ps_big")
                n_done = {id(p): 0 for p in psvs + [psq, psk]}
                TOT = 3 * T2

                def emit(ps, lhsT, rhs):
                    n = n_done[id(ps)]
                    nc.tensor.matmul(
                        ps[:], lhsT=lhsT, rhs=rhs,
                        start=(n == 0), stop=(n == TOT - 1), perf_mode=DR,
                    )
                    n_done[id(ps)] = n + 1

                for s in range(T2 + 2):
                    for pi, t2 in ((0, s), (1, s - 1), (2, s - 2)):
                        if not (0 <= t2 < T2):
                            continue
                        xh_v = xth[t2][:].rearrange("p (j s) -> p j s", j=2)
                        xl_v = xtl[t2][:].rearrange("p (j s) -> p j s", j=2)
                        xv = (xh_v, xh_v, xl_v)[pi]
                        wv_v = (wvh_v, wvl_v, wvh_v)[pi]
                        for st, psv in zip(sts, psvs):
                            emit(psv, xv[:, :, st * 128 : (st + 1) * 128], wv_v[:, t2])
                        sbs = slice(i * 512, (i + 1) * 512)
                        emit(psq, ((wqh_v, wql_v, wqh_v)[pi])[:, t2], xv[:, :, sbs])
                        emit(psk, ((wkh_v, wkl_v, wkh_v)[pi])[:, t2], xv[:, :, sbs])

                for st, psv in zip(sts, psvs):
                    v4t = v4_pool.tile([128, 520], BF16, tag=f"v4_{st}")
                    nc.vector.tensor_copy(v4t[:], bvq[:, 0:520])
                    dst = v4t[:].rearrange("p (q c) -> p q c", q=4)[:, :, 0:128]
                    src = psv[:].rearrange("p (q c) -> p q c", q=4)
                    nc.vector.tensor_add(dst, dst, src)
                    v4s[0].append(v4t)
                nc.vector.tensor_scalar_add(
                    qT[:, i * 512 : (i + 1) * 512], psq[:], bq[:, 0:1]
                )
                nc.vector.tensor_scalar_add(
                    kT[:, i * 512 : (i + 1) * 512], psk[:], bk[:, 0:1]
                )

            attn_ps = {}  # (h, qb) -> [(p_tile, off)]

            def attn_scores(h, qb, lo=0, hi=None):
                """Scores + exp (+causal mask) for q-block qb of head h.

                Diagonal k-tiles are trimmed to their live width: tile kt
                covers q_local in [off, 512) with off = max(kt*128-qb*512, 0).
                """
                qT, kT = qks[h][0], qks[h][1]
                n_kt = 4 * qb + 4
                if hi is None:
                    hi = n_kt
                ps = attn_ps.setdefault((h, qb), [])
                for kt in range(lo, hi):
                    r = kt * 128 - qb * 512
                    off = max(r, 0)
                    nw = 512 - off
                    pss = ps_big.tile([128, 512], F32, tag="ps_big")
                    nc.tensor.matmul(
                        pss[:, 0:nw],
                        lhsT=kT[:, kt * 128 : (kt + 1) * 128],
                        rhs=qT[:, qb * 512 + off : (qb + 1) * 512],
                        start=True,
                        stop=True,
                    )
                    p_sb = p_pool.tile([128, 512], BF16, tag="p")
                    nc.scalar.activation(
                        p_sb[:, 0:nw], pss[:, 0:nw],
                        mybir.ActivationFunctionType.Exp, scale=SCALE,
                    )
                    if r >= 0:  # diagonal block: apply causal 0/1 mask
                        nc.vector.tensor_mul(
                            p_sb[:, 0:nw], p_sb[:, 0:nw], mask[:, 384 : 384 + nw]
                        )
                    ps.append((p_sb, off))

            def attn_out(h, qb, js=range(4), pop=True):
                """attn @ [v|1], normalize, and store, for q-block qb."""
                g, hq = h // 4, h % 4
                v4 = v4s[g]
                ps = attn_ps[(h, qb)]
                if pop:
                    del attn_ps[(h, qb)]
                for j in js:
                    poj = ps_out.tile([128, 129], F32, tag="po")
                    last_kt = 4 * qb + j  # causality: kt*128 <= qb*512 + j*128
                    for kt in range(last_kt + 1):
                        p_sb, off = ps[kt]
                        nc.tensor.matmul(
                            poj[:],
                            lhsT=p_sb[:, j * 128 - off : j * 128 - off + 128],
                            rhs=v4[kt][:, hq * 130 : hq * 130 + 129],
                            start=(kt == 0),
                            stop=(kt == last_kt),
                        )
                    recip = r_pool.tile([128, 1], F32, tag="recip")
                    nc.vector.reciprocal(recip[:], poj[:, 128:129])
                    o_sb = o_pool.tile([128, 128], F32, tag="o")
                    nc.vector.tensor_scalar_mul(o_sb[:], poj[:, 0:128], recip[:])
                    nc.sync.dma_start(
                        out=out_d.ap()[
                            qb * 512 + j * 128 : qb * 512 + (j + 1) * 128,
                            h * 128 : (h + 1) * 128,
                        ],
                        in_=o_sb[:],
                    )

            # ---- software-pipelined emission ----
            # Each step pairs a PE-dense item (4 chunks) with the attention of
            # an already-projected head: scores(qb) -> pe chunk -> out(qb), so
            # exps for qb run on ACT while PE does projection matmuls.
            def v_item(g):
                v_start(g)
                return [lambda i=i: v_chunk(g, range(4 * i, 4 * i + 4))
                        for i in range(4)]

            def qkv_item(h):
                qkv_start(h)
                return [lambda sb=sb: qkv_chunk(h, sb) for sb in range(NQB)]

            def merged_item():
                v_start(0)
                qkv_start(0)
                return [lambda i=i: merged_chunk(i) for i in range(4)]

            steps = [
                (lambda: v_item(0), None),
                (lambda: qkv_item(0), None),
                (lambda: qkv_item(1), 0),
                (lambda: qkv_item(2), 1),
                (lambda: qkv_item(3), 2),
                (lambda: qkv_item(4), 3),
                (lambda: v_item(1), None),
                (lambda: qkv_item(5), 4),
                (lambda: qkv_item(6), 5),
                (lambda: qkv_item(7), 6),
            ]
            for step_i, (pe_item, h_attn) in enumerate(steps):
                pe_chunks = pe_item() if pe_item is not None else [None] * NQB
                _lo = {0: 0, 1: 2, 2: 4, 3: 5}
                for i in range(NQB):
                    if h_attn is not None:
                        attn_scores(h_attn, i, lo=_lo[i])
                    if pe_chunks[i] is not None:
                        pe_chunks[i]()
                    if h_attn is not None:
                        if i + 1 < NQB and _lo[i + 1] > 0:
                            attn_scores(h_attn, i + 1, lo=0, hi=_lo[i + 1])
                        attn_out(h_attn, i)
                if step_i == len(steps) - 1:
                    # pull qb 0/1 of the last head into this step so the final
                    # tail is not ACT-bound: these exps overlap av of h=6
                    for qb in (0, 1):
                        attn_scores(7, qb)
                        attn_out(7, qb)
            # remaining tail: pipeline qb3's last exps against per-j av so the
            # final ACT-to-PE handoff is as short as possible
            attn_scores(7, 2)
            attn_scores(7, 3, lo=0, hi=13)
            attn_out(7, 2)
            for j in range(4):
                if j < 3:
                    attn_scores(7, 3, lo=13 + j, hi=14 + j)
                attn_out(7, 3, js=[j], pop=(j == 3))
    nc.compile()
    return nc


def _hilo(x):
    hi = x.astype(F8_NP)
    lo = (x - hi.astype(np.float32)).astype(F8_NP)
    return hi, lo


def make_in_maps(hidden_states, W, b):
    """Host-side sharding: slice/transpose/cast inputs per core."""
    X = np.asarray(hidden_states, dtype=np.float32)
    Wf = np.asarray(W, dtype=np.float32).reshape(D, D, 3) * WS
    bf = np.asarray(b, dtype=np.float32).reshape(D, 3) * WS

    # causal staircase mask: mask[p, c] = 1 if c >= p + 384
    cols = np.arange(896)[None, :]
    rows = np.arange(128)[:, None]
    mask = (cols >= rows + 384).astype(BF16_NP)

    in_maps = []
    for c in range(N_CORES):
        bcore, hg = c // 2, c % 2
        dm0 = hg * 1024
        # X^T planes in dt-pair layout [t2][p][j*S+s]
        xt_f = np.ascontiguousarray(X[bcore].T)  # [D, S] f32
        xh, xl = _hilo(xt_f)
        xth = np.ascontiguousarray(
            xh.reshape(T2, 2, 128, S).transpose(0, 2, 1, 3)
        ).reshape(T2, 128, 2 * S)
        xtl = np.ascontiguousarray(
            xl.reshape(T2, 2, 128, S).transpose(0, 2, 1, 3)
        ).reshape(T2, 128, 2 * S)

        def w_planes(col_plane):
            # [D, 1024] -> [8 heads][128 p][t2*256 + j*128 + m]
            wh, wl = _hilo(col_plane)
            def lay(a):
                return np.ascontiguousarray(
                    a.reshape(T2, 2, 128, 8, 128).transpose(3, 2, 0, 1, 4)
                ).reshape(8, 128, DT * 128)
            return lay(wh), lay(wl)

        wqh, wql = w_planes(Wf[:, dm0 : dm0 + 1024, 0])
        wkh, wkl = w_planes(Wf[:, dm0 : dm0 + 1024, 2])

        # V: [D, 1024] -> [2 groups][128 p][t2*1024 + j*512 + c]
        vh, vl = _hilo(Wf[:, dm0 : dm0 + 1024, 1])
        def v_lay(a):
            return np.ascontiguousarray(
                a.reshape(T2, 2, 128, 2, 512).transpose(3, 2, 0, 1, 4)
            ).reshape(2, 128, DT * 512)
        wvh, wvl = v_lay(vh), v_lay(vl)

        bq = np.ascontiguousarray(
            bf[dm0 : dm0 + 1024, 0].reshape(8, 128).T
        ).astype(np.float32)
        bk = np.ascontiguousarray(
            bf[dm0 : dm0 + 1024, 2].reshape(8, 128).T
        ).astype(np.float32)
        bv = bf[dm0 : dm0 + 1024, 1].reshape(2, 4, 128)
        bvq = np.zeros((2, 128, 520), dtype=BF16_NP)
        for g in range(2):
            for hq in range(4):
                bvq[g, :, hq * 130 : hq * 130 + 128] = bv[g, hq][None, :].astype(
                    BF16_NP
                )
                bvq[g, :, hq * 130 + 128] = BF16_NP(WS)  # rowsum col: x128 like v
        in_maps.append(
            {
                "xth": xth, "xtl": xtl,
                "wqh": wqh, "wql": wql, "wkh": wkh, "wkl": wkl,
                "wvh": wvh, "wvl": wvl,
                "bq": bq, "bk": bk, "bvq": bvq, "mask": mask,
            }
        )
    return in_maps


def gather_out(results):
    out = np.empty((B, S, D), dtype=np.float32)
    for c in range(N_CORES):
        bcore, hg = c // 2, c % 2
        out[bcore][:, hg * 1024 : hg * 1024 + 1024] = results[c]["out"]
    return out


_CACHED_NC = None


def kernel(hidden_states, W, b):
    global _CACHED_NC
    if _CACHED_NC is None:
        _CACHED_NC = build_nc()
    in_maps = make_in_maps(hidden_states, W, b)
    res = run_bass_kernel_spmd(_CACHED_NC, in_maps, core_ids=list(range(N_CORES)))
    return gather_out(res.results)


# revision 36
# speedup vs baseline: 1.2078x; 1.0203x over previous
"""Bloom self-attention (fused QKV + causal softmax attention) on 8 TRN2 cores.

Sharding: core c handles batch b=c//2 and head-group hg=c%2 (8 of 16 heads).
Each core computes QKV projection for its columns of W, then causal attention
for its 8 heads, writing out[s, 1024] (fp32). Host transposes/casts/slices
inputs and gathers outputs.

v2: QKV projections run as fp8(e4m3) DoubleRow matmuls — 2 contraction
k-tiles per instruction at 0.5 cycles/row, 4x bf16 PE throughput per pass.
To reach bf16-level accuracy each operand is split into fp8 hi+lo planes and
three passes accumulate (hi*hi + lo*hi + hi*lo); W is pre-scaled by 128 on
the host so the W lo-plane stays above the fp8 denormal threshold (raw
|W|~0.02 residuals would flush to zero). The scale cancels: q,k are 128x
(scores exp scale absorbs 128^2), v is 128x and the rowsum ones-column is
128.0 so normalization divides it out. Attention (scores/exp/mask/attn@v)
stays bf16, identical to the baseline structure.

Layout notes (per core, on device):
  xth/xtl [8,128,2*2048] fp8 : X^T hi/lo planes, dt-pair tiles
                               [p, j*2048+s] = plane(X[s, (2t2+j)*128+p])
  wqh/wql/wkh/wkl [8,128,2048] fp8 : per head, [p, t2*256+j*128+m]
  wvh/wvl [2,128,8192] fp8   : per group, [p, t2*1024+j*512+c]
  bq/bk  [128,8] f32         : per-partition bias columns per head (x128)
  bvq    [2,128,520] bf16    : v-bias rows (x128) + 128.0 ones columns
  mask   [128,896] bf16      : causal staircase (as baseline)
"""

import math
from contextlib import ExitStack

import numpy as np
import ml_dtypes

import concourse.mybir as mybir
import concourse.tile as tile
from concourse import bacc
from concourse.bass_utils import run_bass_kernel_spmd

B, S, D = 4, 2048, 2048
H, HD = 16, 128
N_CORES = 8
DT = D // 128   # 16 d-tiles
T2 = DT // 2    # 8 dt-pairs for DoubleRow
NQB = S // 512  # 4 q-blocks
WS = 128.0      # host pre-scale on W/b (keeps fp8 lo-planes out of denormals)
SCALE = 1.0 / (math.sqrt(HD) * WS * WS)

BF16 = mybir.dt.bfloat16
F8 = mybir.dt.float8e4
F32 = mybir.dt.float32
BF16_NP = ml_dtypes.bfloat16
F8_NP = ml_dtypes.float8_e4m3
DR = mybir.MatmulPerfMode.DoubleRow


def build_nc(repeat: int = 1):
    nc = bacc.Bacc(
        "TRN2",
        target_bir_lowering=False,
        debug=False,
        enable_asserts=False,
        num_devices=N_CORES,
    )
    xth_d = nc.dram_tensor("xth", [T2, 128, 2 * S], F8, kind="ExternalInput")
    xtl_d = nc.dram_tensor("xtl", [T2, 128, 2 * S], F8, kind="ExternalInput")
    wqh_d = nc.dram_tensor("wqh", [8, 128, DT * 128], F8, kind="ExternalInput")
    wql_d = nc.dram_tensor("wql", [8, 128, DT * 128], F8, kind="ExternalInput")
    wkh_d = nc.dram_tensor("wkh", [8, 128, DT * 128], F8, kind="ExternalInput")
    wkl_d = nc.dram_tensor("wkl", [8, 128, DT * 128], F8, kind="ExternalInput")
    wvh_d = nc.dram_tensor("wvh", [2, 128, DT * 512], F8, kind="ExternalInput")
    wvl_d = nc.dram_tensor("wvl", [2, 128, DT * 512], F8, kind="ExternalInput")
    bq_d = nc.dram_tensor("bq", [128, 8], F32, kind="ExternalInput")
    bk_d = nc.dram_tensor("bk", [128, 8], F32, kind="ExternalInput")
    bvq_d = nc.dram_tensor("bvq", [2, 128, 520], BF16, kind="ExternalInput")
    mask_d = nc.dram_tensor("mask", [128, 896], BF16, kind="ExternalInput")
    out_d = nc.dram_tensor("out", [S, 1024], F32, kind="ExternalOutput")

    with ExitStack() as ctx:
        tc = ctx.enter_context(tile.TileContext(nc))
        singles = ctx.enter_context(tc.tile_pool(name="singles", bufs=1))
        wqk_pool = ctx.enter_context(tc.tile_pool(name="wqk", bufs=2))
        wv_pool = ctx.enter_context(tc.tile_pool(name="wvp", bufs=2))
        qk_pool = ctx.enter_context(tc.tile_pool(name="qk", bufs=2))
        v4_pool = ctx.enter_context(tc.tile_pool(name="v4", bufs=2))
        p_pool = ctx.enter_context(tc.tile_pool(name="pp", bufs=32))
        o_pool = ctx.enter_context(tc.tile_pool(name="op", bufs=4))
        r_pool = ctx.enter_context(tc.tile_pool(name="rp", bufs=8))
        ps_big = ctx.enter_context(tc.tile_pool(name="ps_big", bufs=8, space="PSUM"))

        # ---- resident constants (loaded once) ----
        # Interleave wv(0) chunk DMAs with the xt tile DMAs: the first
        # (t2-outer) v-chunk consumes exactly wv[:, t2*1024:...]+xt[t2] per
        # step, so the PE ramp starts as soon as the first pair lands.
        wvh_first = wv_pool.tile([128, DT * 512], F8, tag="wvh")
        wvl_first = wv_pool.tile([128, DT * 512], F8, tag="wvl")
        xth = []
        xtl = []
        for t2 in range(T2):
            nc.sync.dma_start(
                out=wvh_first[:, t2 * 1024 : (t2 + 1) * 1024],
                in_=wvh_d.ap()[0, :, t2 * 1024 : (t2 + 1) * 1024],
            )
            nc.sync.dma_start(
                out=wvl_first[:, t2 * 1024 : (t2 + 1) * 1024],
                in_=wvl_d.ap()[0, :, t2 * 1024 : (t2 + 1) * 1024],
            )
            th = singles.tile([128, 2 * S], F8, tag=f"xth{t2}")
            nc.sync.dma_start(out=th[:], in_=xth_d.ap()[t2, :, :])
            xth.append(th)
            tl = singles.tile([128, 2 * S], F8, tag=f"xtl{t2}")
            nc.sync.dma_start(out=tl[:], in_=xtl_d.ap()[t2, :, :])
            xtl.append(tl)
        mask = singles.tile([128, 896], BF16, tag="mask")
        nc.sync.dma_start(out=mask[:], in_=mask_d.ap())
        bvq = singles.tile([128, 2 * 520], BF16, tag="bvq")
        for g in range(2):
            nc.sync.dma_start(
                out=bvq[:, g * 520 : (g + 1) * 520], in_=bvq_d.ap()[g, :, :]
            )
        bq = singles.tile([128, 8], F32, tag="bq")
        nc.sync.dma_start(out=bq[:], in_=bq_d.ap())
        bk = singles.tile([128, 8], F32, tag="bk")
        nc.sync.dma_start(out=bk[:], in_=bk_d.ap())
        # prewarm the ACT exp table set (~2.7us PSEUDO_LOAD on first Exp)
        warm = singles.tile([128, 1], F32, tag="warm")
        nc.vector.memset(warm[:], 0.0)
        nc.scalar.activation(warm[:], warm[:], mybir.ActivationFunctionType.Exp)


        def dr3(psum, xh_ap, xl_ap, wh_ap, wl_ap, t2, x_stationary):
            """Emit the 3 compensated fp8 DR passes for one t2 pair.

            x_stationary: X planes are lhsT (v-projection); else W planes
            are lhsT (q/k projection)."""
            first = t2 == 0
            last = t2 == T2 - 1
            if x_stationary:
                triples = [(xh_ap, wh_ap), (xh_ap, wl_ap), (xl_ap, wh_ap)]
            else:
                triples = [(wh_ap, xh_ap), (wl_ap, xh_ap), (wh_ap, xl_ap)]
            for i, (lhsT, rhs) in enumerate(triples):
                nc.tensor.matmul(
                    psum,
                    lhsT=lhsT,
                    rhs=rhs,
                    start=(first and i == 0),
                    stop=(last and i == 2),
                    perf_mode=DR,
                )

        for _rep in range(repeat):
            # per-rep state: tiles keyed by quad / head
            v4s = {}     # g -> [16 v4 tiles]
            wv_gs = {}   # g -> (wvh, wvl) tiles
            qks = {}     # h -> (qT, kT, (wqh, wql, wkh, wkl))

            def v_start(g):
                if g == 0 and _rep == 0:
                    wv_g = (wvh_first, wvl_first)
                else:
                    wvh_g = wv_pool.tile([128, DT * 512], F8, tag="wvh")
                    nc.sync.dma_start(out=wvh_g[:], in_=wvh_d.ap()[g, :, :])
                    wvl_g = wv_pool.tile([128, DT * 512], F8, tag="wvl")
                    nc.sync.dma_start(out=wvl_g[:], in_=wvl_d.ap()[g, :, :])
                    wv_g = (wvh_g, wvl_g)
                wv_gs[g] = wv_g
                v4s[g] = []

            def v_chunk(g, sts):
                """v4[st] = X @ Wv_quad + bv (+ interleaved ones cols).

                t2-outer over the st group so each xt[t2] tile is consumed as
                soon as its DMA lands (matters for the startup ramp)."""
                wvh_g, wvl_g = wv_gs[g]
                wvh_v = wvh_g[:].rearrange("p (t j c) -> p t j c", t=T2, j=2)
                wvl_v = wvl_g[:].rearrange("p (t j c) -> p t j c", t=T2, j=2)
                sts = list(sts)
                psvs = []
                for st in sts:
                    psv = ps_big.tile([128, 512], F32, tag="ps_big")
                    psvs.append(psv)
                for t2 in range(T2):
                    xh_v = xth[t2][:].rearrange("p (j s) -> p j s", j=2)
                    xl_v = xtl[t2][:].rearrange("p (j s) -> p j s", j=2)
                    for st, psv in zip(sts, psvs):
                        dr3(
                            psv[:],
                            xh_v[:, :, st * 128 : (st + 1) * 128],
                            xl_v[:, :, st * 128 : (st + 1) * 128],
                            wvh_v[:, t2],
                            wvl_v[:, t2],
                            t2,
                            x_stationary=True,
                        )
                for st, psv in zip(sts, psvs):
                    v4t = v4_pool.tile([128, 520], BF16, tag=f"v4_{st}")
                    nc.vector.tensor_copy(v4t[:], bvq[:, g * 520 : (g + 1) * 520])
                    dst = v4t[:].rearrange("p (q c) -> p q c", q=4)[:, :, 0:128]
                    src = psv[:].rearrange("p (q c) -> p q c", q=4)
                    nc.vector.tensor_add(dst, dst, src)
                    v4s[g].append(v4t)

            def qkv_start(h):
                ws = []
                for nm, d in (
                    ("wqh", wqh_d), ("wql", wql_d),
                    ("wkh", wkh_d), ("wkl", wkl_d),
                ):
                    t = wqk_pool.tile([128, DT * 128], F8, tag=nm)
                    nc.sync.dma_start(out=t[:], in_=d.ap()[h, :, :])
                    ws.append(t)
                ws = tuple(ws)
                qT = qk_pool.tile([128, S], BF16, tag="qT")
                kT = qk_pool.tile([128, S], BF16, tag="kT")
                qks[h] = (qT, kT, ws)

            def qkv_chunk(h, sb):
                """qT/kT columns for s-block sb of head h."""
                qT, kT, (wqh_h, wql_h, wkh_h, wkl_h) = qks[h]
                for wh, wl, dest, bias in (
                    (wqh_h, wql_h, qT, bq),
                    (wkh_h, wkl_h, kT, bk),
                ):
                    wh_v = wh[:].rearrange("p (t j m) -> p t j m", t=T2, j=2)
                    wl_v = wl[:].rearrange("p (t j m) -> p t j m", t=T2, j=2)
                    psx = ps_big.tile([128, 512], F32, tag="ps_big")
                    for t2 in range(T2):
                        xh_v = xth[t2][:].rearrange("p (j s) -> p j s", j=2)
                        xl_v = xtl[t2][:].rearrange("p (j s) -> p j s", j=2)
                        dr3(
                            psx[:],
                            xh_v[:, :, sb * 512 : (sb + 1) * 512],
                            xl_v[:, :, sb * 512 : (sb + 1) * 512],
                            wh_v[:, t2],
                            wl_v[:, t2],
                            t2,
                            x_stationary=False,
                        )
                    nc.vector.tensor_scalar_add(
                        dest[:, sb * 512 : (sb + 1) * 512], psx[:], bias[:, h : h + 1]
                    )

            def merged_chunk(i):
                """Startup chunk: v0 sts 4i..4i+3 + qkv0 (q,k) for sb=i.

                Passes are slot-pipelined across t2 (A at slot s, C at s-1,
                B at s-2) so consumption tracks the per-t2 DMA delivery
                order (wvh, xth, wvl, xtl) with only 6 open PSUM groups."""
                wvh_g, wvl_g = wv_gs[0]
                wvh_v = wvh_g[:].rearrange("p (t j c) -> p t j c", t=T2, j=2)
                wvl_v = wvl_g[:].rearrange("p (t j c) -> p t j c", t=T2, j=2)
                qT, kT, (wqh_h, wql_h, wkh_h, wkl_h) = qks[0]
                wqh_v = wqh_h[:].rearrange("p (t j m) -> p t j m", t=T2, j=2)
                wql_v = wql_h[:].rearrange("p (t j m) -> p t j m", t=T2, j=2)
                wkh_v = wkh_h[:].rearrange("p (t j m) -> p t j m", t=T2, j=2)
                wkl_v = wkl_h[:].rearrange("p (t j m) -> p t j m", t=T2, j=2)
                sts = list(range(4 * i, 4 * i + 4))
                psvs = []
                for _st in sts:
                    psv = ps_big.tile([128, 512], F32, tag="ps_big")
                    psvs.append(psv)
                psq = ps_big.tile([128, 512], F32, tag="ps_big")
                psk = ps_big.tile([128, 512], F32, tag="ps_big")
                n_done = {id(p): 0 for p in psvs + [psq, psk]}
                TOT = 3 * T2

                def emit(ps, lhsT, rhs):
                    n = n_done[id(ps)]
                    nc.tensor.matmul(
                        ps[:], lhsT=lhsT, rhs=rhs,
                        start=(n == 0), stop=(n == TOT - 1), perf_mode=DR,
                    )
                    n_done[id(ps)] = n + 1

                for s in range(T2 + 2):
                    for pi, t2 in ((0, s), (1, s - 1), (2, s - 2)):
                        if not (0 <= t2 < T2):
                            continue
                        xh_v = xth[t2][:].rearrange("p (j s) -> p j s", j=2)
                        xl_v = xtl[t2][:].rearrange("p (j s) -> p j s", j=2)
                        xv = (xh_v, xh_v, xl_v)[pi]
                        wv_v = (wvh_v, wvl_v, wvh_v)[pi]
                        for st, psv in zip(sts, psvs):
                            emit(psv, xv[:, :, st * 128 : (st + 1) * 128], wv_v[:, t2])
                        sbs = slice(i * 512, (i + 1) * 512)
                        emit(psq, ((wqh_v, wql_v, wqh_v)[pi])[:, t2], xv[:, :, sbs])
                        emit(psk, ((wkh_v, wkl_v, wkh_v)[pi])[:, t2], xv[:, :, sbs])

                for st, psv in zip(sts, psvs):
                    v4t = v4_pool.tile([128, 520], BF16, tag=f"v4_{st}")
                    nc.vector.tensor_copy(v4t[:], bvq[:, 0:520])
                    dst = v4t[:].rearrange("p (q c) -> p q c", q=4)[:, :, 0:128]
                    src = psv[:].rearrange("p (q c) -> p q c", q=4)
                    nc.vector.tensor_add(dst, dst, src)
                    v4s[0].append(v4t)
                nc.vector.tensor_scalar_add(
                    qT[:, i * 512 : (i + 1) * 512], psq[:], bq[:, 0:1]
                )
                nc.vector.tensor_scalar_add(
                    kT[:, i * 512 : (i + 1) * 512], psk[:], bk[:, 0:1]
                )

            attn_ps = {}  # (h, qb) -> [(p_tile, off)]

            def attn_scores(h, qb, lo=0, hi=None):
                """Scores + exp (+causal mask) for q-block qb of head h.

                Diagonal k-tiles are trimmed to their live width: tile kt
                covers q_local in [off, 512) with off = max(kt*128-qb*512, 0).
                """
                qT, kT = qks[h][0], qks[h][1]
                n_kt = 4 * qb + 4
                if hi is None:
                    hi = n_kt
                ps = attn_ps.setdefault((h, qb), [])
                for kt in range(lo, hi):
                    r = kt * 128 - qb * 512
                    off = max(r, 0)
                    nw = 512 - off
                    pss = ps_big.tile([128, 512], F32, tag="ps_big")
                    nc.tensor.matmul(
                        pss[:, 0:nw],
                        lhsT=kT[:, kt * 128 : (kt + 1) * 128],
                        rhs=qT[:, qb * 512 + off : (qb + 1) * 512],
                        start=True,
                        stop=True,
                    )
                    p_sb = p_pool.tile([128, 512], BF16, tag="p")
                    nc.scalar.activation(
                        p_sb[:, 0:nw], pss[:, 0:nw],
                        mybir.ActivationFunctionType.Exp, scale=SCALE,
                    )
                    if r >= 0:  # diagonal block: apply causal 0/1 mask
                        nc.vector.tensor_mul(
                            p_sb[:, 0:nw], p_sb[:, 0:nw], mask[:, 384 : 384 + nw]
                        )
                    ps.append((p_sb, off))

            def attn_out(h, qb, js=range(4), pop=True):
                """attn @ [v|1], normalize, and store, for q-block qb."""
                g, hq = h // 4, h % 4
                v4 = v4s[g]
                ps = attn_ps[(h, qb)]
                if pop:
                    del attn_ps[(h, qb)]
                for j in js:
                    poj_t = ps_big.tile([128, 512], F32, tag="ps_big")
                    poj = poj_t[:, 0:129]
                    last_kt = 4 * qb + j  # causality: kt*128 <= qb*512 + j*128
                    for kt in range(last_kt + 1):
                        p_sb, off = ps[kt]
                        nc.tensor.matmul(
                            poj,
                            lhsT=p_sb[:, j * 128 - off : j * 128 - off + 128],
                            rhs=v4[kt][:, hq * 130 : hq * 130 + 129],
                            start=(kt == 0),
                            stop=(kt == last_kt),
                        )
                    recip = r_pool.tile([128, 1], F32, tag="recip")
                    nc.vector.reciprocal(recip[:], poj_t[:, 128:129])
                    o_sb = o_pool.tile([128, 128], F32, tag="o")
                    nc.vector.tensor_scalar_mul(o_sb[:], poj_t[:, 0:128], recip[:])
                    nc.sync.dma_start(
                        out=out_d.ap()[
                            qb * 512 + j * 128 : qb * 512 + (j + 1) * 128,
                            h * 128 : (h + 1) * 128,
                        ],
                        in_=o_sb[:],
                    )

            # ---- software-pipelined emission ----
            # Each step pairs a PE-dense item (4 chunks) with the attention of
            # an already-projected head: scores(qb) -> pe chunk -> out(qb), so
            # exps for qb run on ACT while PE does projection matmuls.
            def v_item(g):
                v_start(g)
                return [lambda i=i: v_chunk(g, range(4 * i, 4 * i + 4))
                        for i in range(4)]

            def qkv_item(h):
                qkv_start(h)
                return [lambda sb=sb: qkv_chunk(h, sb) for sb in range(NQB)]

            def merged_item():
                v_start(0)
                qkv_start(0)
                return [lambda i=i: merged_chunk(i) for i in range(4)]

            steps = [
                (lambda: v_item(0), None),
                (lambda: qkv_item(0), None),
                (lambda: qkv_item(1), 0),
                (lambda: qkv_item(2), 1),
                (lambda: qkv_item(3), 2),
                (lambda: qkv_item(4), 3),
                (lambda: v_item(1), None),
                (lambda: qkv_item(5), 4),
                (lambda: qkv_item(6), 5),
                (lambda: qkv_item(7), 6),
            ]
            for step_i, (pe_item, h_attn) in enumerate(steps):
                pe_chunks = pe_item() if pe_item is not None else [None] * NQB
                _lo = {0: 0, 1: 2, 2: 4, 3: 5}
                for i in range(NQB):
                    if h_attn is not None:
                        attn_scores(h_attn, i, lo=_lo[i])
                    if pe_chunks[i] is not None:
                        pe_chunks[i]()
                    if h_attn is not None:
                        if i + 1 < NQB and _lo[i + 1] > 0:
                            attn_scores(h_attn, i + 1, lo=0, hi=_lo[i + 1])
                        attn_out(h_attn, i)
                if step_i == len(steps) - 1:
                    # pull qb 0/1 of the last head into this step so the final
                    # tail is not ACT-bound: these exps overlap av of h=6
                    for qb in (0, 1):
                        attn_scores(7, qb)
                        attn_out(7, qb)
            # remaining tail: pipeline qb3's last exps against per-j av so the
            # final ACT-to-PE handoff is as short as possible
            attn_scores(7, 2)
            attn_scores(7, 3, lo=0, hi=13)
            attn_out(7, 2)
            for j in range(4):
                if j < 3:
                    attn_scores(7, 3, lo=13 + j, hi=14 + j)
                attn_out(7, 3, js=[j], pop=(j == 3))
    nc.compile()
    return nc


def _hilo(x):
    hi = x.astype(F8_NP)
    lo = (x - hi.astype(np.float32)).astype(F8_NP)
    return hi, lo


def make_in_maps(hidden_states, W, b):
    """Host-side sharding: slice/transpose/cast inputs per core."""
    X = np.asarray(hidden_states, dtype=np.float32)
    Wf = np.asarray(W, dtype=np.float32).reshape(D, D, 3) * WS
    bf = np.asarray(b, dtype=np.float32).reshape(D, 3) * WS

    # causal staircase mask: mask[p, c] = 1 if c >= p + 384
    cols = np.arange(896)[None, :]
    rows = np.arange(128)[:, None]
    mask = (cols >= rows + 384).astype(BF16_NP)

    in_maps = []
    for c in range(N_CORES):
        bcore, hg = c // 2, c % 2
        dm0 = hg * 1024
        # X^T planes in dt-pair layout [t2][p][j*S+s]
        xt_f = np.ascontiguousarray(X[bcore].T)  # [D, S] f32
        xh, xl = _hilo(xt_f)
        xth = np.ascontiguousarray(
            xh.reshape(T2, 2, 128, S).transpose(0, 2, 1, 3)
        ).reshape(T2, 128, 2 * S)
        xtl = np.ascontiguousarray(
            xl.reshape(T2, 2, 128, S).transpose(0, 2, 1, 3)
        ).reshape(T2, 128, 2 * S)

        def w_planes(col_plane):
            # [D, 1024] -> [8 heads][128 p][t2*256 + j*128 + m]
            wh, wl = _hilo(col_plane)
            def lay(a):
                return np.ascontiguousarray(
                    a.reshape(T2, 2, 128, 8, 128).transpose(3, 2, 0, 1, 4)
                ).reshape(8, 128, DT * 128)
            return lay(wh), lay(wl)

        wqh, wql = w_planes(Wf[:, dm0 : dm0 + 1024, 0])
        wkh, wkl = w_planes(Wf[:, dm0 : dm0 + 1024, 2])

        # V: [D, 1024] -> [2 groups][128 p][t2*1024 + j*512 + c]
        vh, vl = _hilo(Wf[:, dm0 : dm0 + 1024, 1])
        def v_lay(a):
            return np.ascontiguousarray(
                a.reshape(T2, 2, 128, 2, 512).transpose(3, 2, 0, 1, 4)
            ).reshape(2, 128, DT * 512)
        wvh, wvl = v_lay(vh), v_lay(vl)

        bq = np.ascontiguousarray(
            bf[dm0 : dm0 + 1024, 0].reshape(8, 128).T
        ).astype(np.float32)
        bk = np.ascontiguousarray(
            bf[dm0 : dm0 + 1024, 2].reshape(8, 128).T
        ).astype(np.float32)
        bv = bf[dm0 : dm0 + 1024, 1].reshape(2, 4, 128)
        bvq = np.zeros((2, 128, 520), dtype=BF16_NP)
        for g in range(2):
            for hq in range(4):
                bvq[g, :, hq * 130 : hq * 130 + 128] = bv[g, hq][None, :].astype(
                    BF16_NP
                )
                bvq[g, :, hq * 130 + 128] = BF16_NP(WS)  # rowsum col: x128 like v
        in_maps.append(
            {
                "xth": xth, "xtl": xtl,
                "wqh": wqh, "wql": wql, "wkh": wkh, "wkl": wkl,
                "wvh": wvh, "wvl": wvl,
                "bq": bq, "bk": bk, "bvq": bvq, "mask": mask,
            }
        )
    return in_maps


def gather_out(results):
    out = np.empty((B, S, D), dtype=np.float32)
    for c in range(N_CORES):
        bcore, hg = c // 2, c % 2
        out[bcore][:, hg * 1024 : hg * 1024 + 1024] = results[c]["out"]
    return out


_CACHED_NC = None


def kernel(hidden_states, W, b):
    global _CACHED_NC
    if _CACHED_NC is None:
        _CACHED_NC = build_nc()
    in_maps = make_in_maps(hidden_states, W, b)
    res = run_bass_kernel_spmd(_CACHED_NC, in_maps, core_ids=list(range(N_CORES)))
    return gather_out(res.results)
